# revision 24
# baseline (speedup 1.0000x reference)
"""Causal multi-head attention block (B=4, T=2048, C=1024, H=16) on 8 NeuronCores.

Sharding: core c = 2*b + hg handles batch b, head-group hg (8 heads).
Data parallel over B, tensor parallel over heads: qkv weights column-split,
proj weights row-split; each core emits a partial projection output which the
host sums per batch (plus proj bias).

Per-core pipeline (v2): single flat instruction stream that keeps the PE dense.
  - qkv projections produce qT/kT in bf16 [128, 2048] (head dims on
    partitions) and v token-major bf16 with a ones column.
  - attention runs per (head, 512-query quarter): scoresT tiles (bf16
    matmul, K padded 64->128 to keep the PE clock-gate warm), exp on ACT
    over cj-PAIRS packed side by side in one [128,1024] PSUM slab (halves
    the ACT instruction count), causal diag masked by 0/1 bf16 multiply,
    AV accumulates [65, 512] in PSUM where the ones column yields row sums.
  - the scores->exp->AV chain is software-pipelined one unit deep ACROSS
    quarter/head boundaries, and the NEXT pair's qkT projection matmuls are
    interleaved as fill work so the PE never idles on ACT latency.
  - normalization per quarter: reciprocal of row sums via DRAM bounce
    (fold [1,512] across partitions), broadcast, multiply -> yT bf16.
  - output projection in bf16 against row-slice of proj_w -> partial
    [2048, 1024] fp32, host sums pairs + proj bias.
"""
import numpy as np
import ml_dtypes

import concourse.bacc as bacc
import concourse.mybir as mybir
import concourse.tile as tile
from concourse.bass_utils import run_bass_kernel_spmd

B, T, C, H, D = 4, 2048, 1024, 16, 64
NC_CORES = 8
HPC = H // 2          # heads per core = 8
CW = 3 * C // 2       # packed local qkv width = 1536
F32 = mybir.dt.float32
F32R = mybir.dt.float32r
BF16 = mybir.dt.bfloat16

NT = T // 128         # 16 token tiles
NCC = C // 128        # 8 contraction chunks
NQ = 4                # query quarters per head
QW = 512              # quarter width

TRACE = False          # test.py sets True to profile
LAST_RESULT = None     # BassKernelResults of the last run (for test.py)

_DONE = object()       # generator-exhausted sentinel (yields return None)
_cached_nc = None


def _build():
    global _cached_nc
    if _cached_nc is not None:
        return _cached_nc

    nc = bacc.Bacc("TRN2", debug=False)

    xT_d = nc.dram_tensor("xT", [C, T], F32R, kind="ExternalInput")
    w_d = nc.dram_tensor("w", [C, CW], F32R, kind="ExternalInput")
    wp_d = nc.dram_tensor("wp", [C // 2, C], BF16, kind="ExternalInput")
    bqk_d = nc.dram_tensor("bqk", [128, 8], F32, kind="ExternalInput")
    bv_d = nc.dram_tensor("bv", [128, 512], F32, kind="ExternalInput")
    maskadd_d = nc.dram_tensor("maskadd", [128, 128], F32, kind="ExternalInput")
    ident_d = nc.dram_tensor("ident", [128, 128], F32, kind="ExternalInput")
    out_d = nc.dram_tensor("partial", [T, C], BF16, kind="ExternalOutput")

    with tile.TileContext(nc) as tc:
        with (
            tc.tile_pool(name="const", bufs=1) as const,
            tc.tile_pool(name="dramp", bufs=4, space="DRAM") as dramp,
            tc.tile_pool(name="ps", bufs=1, space="PSUM") as psp,
            tc.tile_pool(name="expp", bufs=2) as expp,
            tc.tile_pool(name="nrm", bufs=2) as nrm,
            tc.tile_pool(name="outp", bufs=4) as outp,
        ):
            # small consts first so the warmup matmuls can start immediately
            identf = const.tile([128, 128], F32)
            nc.sync.dma_start(identf[:], ident_d.ap())
            maskaddf = const.tile([128, 128], F32)
            nc.sync.dma_start(maskaddf[:], maskadd_d.ap())
            bqk = const.tile([128, 8], F32)
            nc.sync.dma_start(bqk[:], bqk_d.ap())
            bv = const.tile([128, 512], F32)
            nc.sync.dma_start(bv[:], bv_d.ap())

            # q/k activations, transposed, bf16: row = head dim (2 heads per
            # tile), col = token. jt 0-3 hold q, 4-7 hold k.
            qkT = [const.tile([128, T], BF16, name=f"qkT{j}") for j in range(8)]
            # zero-padded per-head kT staging: scores matmuls run with K=128
            # (half real head dims, half zeros) to keep the PE activity
            # monitor (clock gate) seeing full-K matmuls.
            kpad = [const.tile([128, T], BF16, name=f"kpad{s}") for s in range(2)]
            nc.vector.memset(kpad[0][:].bitcast(F32), 0.0)
            nc.vector.memset(kpad[1][:].bitcast(F32), 0.0)
            # v with ones column, token-major: v_aug[p, tt, h, d]
            v_aug = const.tile([128, NT, HPC, D + 1], BF16)
            nc.vector.memset(v_aug[:, :, :, D:D + 1], 1.0)
            # normalized attention output, bf16, row = head dim (2 heads)
            yT_sb = [const.tile([128, T], BF16, name=f"yT{k}") for k in range(4)]
            wp_sb = const.tile([128, 4, C], BF16)

            # qkv weight tiles: streamed through a 4-deep pool, prefetched
            # one attention pair ahead of their consuming fill matmuls
            wjt = {}

            # ---- mm tags: two [128,512] PSUM accumulators shared by the qkv
            # projections, v, warmup and P4.  scq: two [128,1024] score slabs.
            # yTq: two [65,512] AV accumulators.  2 + 4 + 2 = 8 banks.
            def mm_tile(i, name):
                return psp.tile([128, 512], F32, tag=f"mm{i}", bufs=1, name=name)

            with (
                tc.tile_pool(name="wjtp", bufs=4) as wjtp,
                tc.tile_pool(name="xp", bufs=1) as xp,
            ):
                def fetch_wjt(jt):
                    t = wjtp.tile(
                        [128, NCC, 128], F32R, tag="wjt", bufs=4, name=f"wjt{jt}"
                    )
                    fetch_wjt_split(jt, t)

                # first-needed weights and x, split fine and issued in consume
                # order so no single transfer serializes a queue for long.
                def fetch_wjt_split(jt, t):
                    for s in range(4):
                        nc.sync.dma_start(
                            t[:, 2 * s:2 * s + 2, :],
                            w_d.ap()[2 * s * 128:(2 * s + 2) * 128,
                                     jt * 128:(jt + 1) * 128].rearrange(
                                "(cc p) j -> p cc j", p=128
                            ),
                        )
                    wjt[jt] = t

                wjt0_t = wjtp.tile([128, NCC, 128], F32R, tag="wjt", bufs=4,
                                   name="wjt0")
                wjt4_t = wjtp.tile([128, NCC, 128], F32R, tag="wjt", bufs=4,
                                   name="wjt4")
                fetch_wjt_split(0, wjt0_t)
                xT = [xp.tile([128, T], F32R, name=f"xT{i}") for i in range(NCC)]
                for qtr in range(2):
                    for cc in range(NCC):
                        nc.sync.dma_start(
                            xT[cc][:, qtr * 512:(qtr + 1) * 512],
                            xT_d.ap()[cc * 128:(cc + 1) * 128,
                                      qtr * 512:(qtr + 1) * 512],
                        )
                fetch_wjt_split(4, wjt4_t)
                for qtr in range(2, 4):
                    for cc in range(NCC):
                        nc.sync.dma_start(
                            xT[cc][:, qtr * 512:(qtr + 1) * 512],
                            xT_d.ap()[cc * 128:(cc + 1) * 128,
                                      qtr * 512:(qtr + 1) * 512],
                        )

                # warm the PE clock gate during the DMA wait: fp32 matmuls on
                # the identity tile (no DVE dependency, first DMA only)
                warm_ps = mm_tile(0, "warm")
                for wi in range(36):
                    nc.tensor.matmul(
                        warm_ps[:, 0:128],
                        lhsT=identf[:],
                        rhs=identf[:],
                        start=True,
                        stop=True,
                        skip_group_check=True,
                    )
                ident = const.tile([128, 128], BF16)
                nc.vector.tensor_copy(ident[:], identf[:])
                maskadd = const.tile([128, 128], BF16)
                nc.vector.tensor_copy(maskadd[:], maskaddf[:])

                def qk_tile_steps(jt):
                    """Generator: one PE matmul (or copy flush) per next()."""
                    for half in range(2):
                        pss = [mm_tile(i, f"qk{jt}_{half}_{i}") for i in range(2)]
                        for cc in range(NCC):
                            for i in range(2):
                                tck = half * 2 + i
                                nc.tensor.matmul(
                                    pss[i][:],
                                    lhsT=wjt[jt][:, cc, :],
                                    rhs=xT[cc][:, tck * 512:(tck + 1) * 512],
                                    start=(cc == 0),
                                    stop=(cc == NCC - 1),
                                )
                                yield
                        for i in range(2):
                            tck = half * 2 + i
                            nc.vector.tensor_scalar_add(
                                qkT[jt][:, tck * 512:(tck + 1) * 512],
                                pss[i][:],
                                bqk[:, jt:jt + 1],
                            )
                        yield

                def emit_all(gen):
                    for _ in gen:
                        pass

                # q,k for pair 0 (heads 0,1) directly
                emit_all(qk_tile_steps(0))
                emit_all(qk_tile_steps(4))

                with tc.tile_pool(name="wvp", bufs=1) as wvp:
                    wv = wvp.tile([128, NCC, 512], F32R)
                    for s in range(4):
                        nc.sync.dma_start(
                            wv[:, 2 * s:2 * s + 2, :],
                            w_d.ap()[2 * s * 128:(2 * s + 2) * 128,
                                     1024:1536].rearrange(
                                "(cc p) j -> p cc j", p=128
                            ),
                        )
                    # pair 0's fill weights stream in behind wv
                    fetch_wjt(1)
                    fetch_wjt(5)
                    # v: token-major, fused bias-add + bf16 pack on DVE
                    for tt in range(NT):
                        ps = mm_tile(tt % 2, f"v{tt}")
                        for cc in range(NCC):
                            nc.tensor.matmul(
                                ps[:],
                                lhsT=xT[cc][:, tt * 128:(tt + 1) * 128],
                                rhs=wv[:, cc, :],
                                start=(cc == 0),
                                stop=(cc == NCC - 1),
                            )
                        nc.vector.tensor_add(
                            v_aug[:, tt, :, 0:D],
                            ps[:].rearrange("p (h d) -> p h d", h=HPC),
                            bv[:].rearrange("p (h d) -> p h d", h=HPC),
                        )

                # proj weights arrive during attention
                nc.sync.dma_start(
                    wp_sb[:], wp_d.ap().rearrange("(kc p) n -> p kc n", p=128)
                )

                # ---------------- attention: flat pipelined stream ----------
                pend = [None]  # deferred (av_closures, fin_closure)

                def flush_pend():
                    if pend[0] is not None:
                        avs, fin = pend[0]
                        for av in avs:
                            av()
                        if fin is not None:
                            fin()
                        pend[0] = None

                def make_fin(yTq_t, hh, off, qlo):
                    def fin():
                        sums = nrm.tile([1, QW], F32, tag="sums", bufs=2)
                        nc.vector.tensor_copy(sums[:], yTq_t[D:D + 1, :])
                        ynum = nrm.tile([64, QW], F32, tag="ynum", bufs=3)
                        nc.vector.tensor_copy(ynum[:], yTq_t[0:D, :])
                        s_dram = dramp.tile([1, QW], F32, tag="sd")
                        nc.sync.dma_start(s_dram[:], sums[:])
                        srb = nrm.tile([128, QW // 128], F32, tag="srb", bufs=2)
                        nc.sync.dma_start(
                            srb[:],
                            s_dram[:].rearrange("o (p f) -> (o p) f", p=128),
                        )
                        rcp = nrm.tile([128, QW // 128], F32, tag="rcp", bufs=2)
                        nc.vector.reciprocal(rcp[:], srb[:])
                        r_dram = dramp.tile([1, QW], F32, tag="rd")
                        nc.sync.dma_start(
                            r_dram[:].rearrange("o (p f) -> (o p) f", p=128),
                            rcp[:],
                        )
                        rb = nrm.tile([64, QW], F32, tag="rb", bufs=2)
                        nc.sync.dma_start(
                            rb[:], r_dram[:].to_broadcast((64, QW))
                        )
                        nc.vector.tensor_mul(
                            yT_sb[hh][off:off + 64, qlo:qlo + QW],
                            ynum[:],
                            rb[:],
                        )
                    return fin

                def attention_pair(p, fills):
                    for h in (2 * p, 2 * p + 1):
                        off = 64 * (h % 2)
                        jq = h // 2
                        kp = kpad[h % 2]
                        nc.vector.tensor_copy(
                            kp[off:off + 64, :], qkT[4 + h // 2][off:off + 64, :]
                        )
                        for q in range(NQ):
                            qlo = q * QW
                            ncj = 4 * q + 4
                            yTq_t = psp.tile(
                                [D + 1, QW], F32, tag="yTq", bufs=2,
                                name=f"yTq{h}_{q}",
                            )
                            fin = make_fin(yTq_t, h // 2, off, qlo)
                            for cjA in range(0, ncj, 2):
                                # pack the cj pair side by side in one slab:
                                # each matmul output stays inside one PSUM bank
                                blocks = []
                                pack = 0
                                for cj in (cjA, cjA + 1):
                                    i0 = cj * 128
                                    s0 = max(i0, qlo)
                                    wdt = qlo + QW - s0
                                    if cj == cjA + 1 and pack == 512 and wdt == 512:
                                        pk = 512
                                    else:
                                        pk = pack if pack % 512 == 0 or pack + wdt <= 512 else 512
                                    blocks.append((cj, i0, s0, wdt, pk))
                                    pack = pk + wdt
                                scq = psp.tile(
                                    [128, 1024], F32, tag="scq", bufs=2,
                                    name=f"scq{h}_{q}_{cjA}",
                                )
                                expT = expp.tile(
                                    [128, 1024], BF16, tag="expT", bufs=2,
                                    name=f"expT{h}_{q}_{cjA}",
                                )
                                for cj, i0, s0, wdt, pk in blocks:
                                    diag = i0 >= qlo
                                    nc.tensor.matmul(
                                        scq[:, pk:pk + wdt],
                                        lhsT=kp[:, i0:i0 + 128],
                                        rhs=qkT[jq][:, s0:s0 + wdt],
                                        start=True,
                                        stop=not diag,
                                        skip_group_check=True,
                                    )
                                    if diag:
                                        # causal mask folded into the PSUM
                                        # group: += maskadd^T (-1e30 above
                                        # diag), exp then yields exact zeros
                                        nc.tensor.matmul(
                                            scq[:, pk:pk + 128],
                                            lhsT=maskadd[:],
                                            rhs=ident[:],
                                            start=False,
                                            stop=True,
                                            skip_group_check=True,
                                        )
                                # fill: next pair's qkv projection matmuls
                                for _ in range(2):
                                    if fills is not None:
                                        if next(fills, _DONE) is _DONE:
                                            fills = None
                                # exp per block (not one wide op): AV of
                                # block A then gates only on exp_A, halving
                                # the exposed ACT latency in the 1-deep pipe
                                for cj, i0, s0, wdt, pk in blocks:
                                    nc.scalar.activation(
                                        expT[:, pk:pk + wdt],
                                        scq[:, pk:pk + wdt],
                                        mybir.ActivationFunctionType.Exp,
                                    )
                                flush_pend()
                                avs = []
                                for cj, i0, s0, wdt, pk in blocks:
                                    def av(cj=cj, s0=s0, wdt=wdt, pk=pk,
                                           expT=expT, yTq_t=yTq_t, h=h,
                                           qlo=qlo, ncj=ncj):
                                        nc.tensor.matmul(
                                            yTq_t[:, s0 - qlo:s0 - qlo + wdt],
                                            lhsT=v_aug[:, cj, h, :],
                                            rhs=expT[:, pk:pk + wdt],
                                            start=(cj == 0),
                                            stop=(cj == ncj - 1),
                                            skip_group_check=True,
                                        )
                                    avs.append(av)
                                pend[0] = (
                                    avs,
                                    fin if cjA == ncj - 2 else None,
                                )
                    return fills

                def qk_pair_fills(p):
                    def chain():
                        yield from qk_tile_steps(p + 1)
                        yield from qk_tile_steps(p + 5)
                    return chain()

                for p in range(3):
                    if p + 2 <= 3:
                        fetch_wjt(p + 2)
                        fetch_wjt(p + 6)
                    leftover = attention_pair(p, qk_pair_fills(p))
                    if leftover is not None:
                        emit_all(leftover)

            # xT freed; pair 3's fill work: P4 partial sums over kc 0-2 for
            # the first 16 output chunks, staged to SBUF (kc3 needs pair 3)
            NPRE = 16
            psb = {}
            with tc.tile_pool(name="psbp", bufs=1) as psbp:
                def p4_prefill_steps():
                    for n in range(NPRE):
                        ci, nck = n // 2, n % 2
                        po = mm_tile(n % 2, f"pop{n}")
                        for ki in range(3):
                            nc.tensor.matmul(
                                po[:],
                                lhsT=yT_sb[ki][:, ci * 128:(ci + 1) * 128],
                                rhs=wp_sb[:, ki, nck * 512:(nck + 1) * 512],
                                start=(ki == 0),
                                stop=(ki == 2),
                            )
                            yield
                        t = psbp.tile([128, 512], F32, name=f"psb{n}")
                        nc.vector.tensor_copy(t[:], po[:])
                        psb[n] = t
                        yield

                leftover = attention_pair(3, p4_prefill_steps())
                if leftover is not None:
                    emit_all(leftover)
                flush_pend()

                # ------------- P4: output projection (partial) -------------
                # prefilled chunks: one kc3 matmul + DVE add of the staged
                # partials; DMA flows immediately
                for n in range(NPRE):
                    ci, nck = n // 2, n % 2
                    po = mm_tile(n % 2, f"pof{n}")
                    nc.tensor.matmul(
                        po[:],
                        lhsT=yT_sb[3][:, ci * 128:(ci + 1) * 128],
                        rhs=wp_sb[:, 3, nck * 512:(nck + 1) * 512],
                        start=True,
                        stop=True,
                    )
                    osb = outp.tile([128, 512], BF16, tag="osb", bufs=4)
                    nc.vector.tensor_add(osb[:], po[:], psb[n][:])
                    nc.sync.dma_start(
                        out_d.ap()[ci * 128:(ci + 1) * 128,
                                   nck * 512:(nck + 1) * 512],
                        osb[:],
                    )
                # remaining chunks: full 4-matmul accumulation, copies on ACT
                # (free after exp) so DVE keeps doing the adds above
                for n in range(NPRE, 2 * NT):
                    ci, nck = n // 2, n % 2
                    po = mm_tile(n % 2, f"po{n}")
                    for ki in range(4):
                        nc.tensor.matmul(
                            po[:],
                            lhsT=yT_sb[ki][:, ci * 128:(ci + 1) * 128],
                            rhs=wp_sb[:, ki, nck * 512:(nck + 1) * 512],
                            start=(ki == 0),
                            stop=(ki == 3),
                        )
                    osb = outp.tile([128, 512], BF16, tag="osb", bufs=4)
                    nc.scalar.copy(osb[:], po[:])
                    nc.sync.dma_start(
                        out_d.ap()[ci * 128:(ci + 1) * 128,
                                   nck * 512:(nck + 1) * 512],
                        osb[:],
                    )

    nc.compile()
    _cached_nc = nc
    return nc


def kernel(x, attn_w, attn_b, proj_w, proj_b):
    global LAST_RESULT
    x = np.asarray(x, dtype=np.float32)
    attn_w = np.asarray(attn_w, dtype=np.float32)
    attn_b = np.asarray(attn_b, dtype=np.float32)
    proj_w = np.asarray(proj_w, dtype=np.float32)
    proj_b = np.asarray(proj_b, dtype=np.float32)

    nc = _build()

    # additive causal mask, pre-transposed for the matmul trick:
    # maskadd[c, p] lands as -1e30 on scoresT[key p, query c] for c < p
    maskadd = np.triu(np.full((128, 128), -1e30, dtype=np.float32), 1)
    ident = np.eye(128, dtype=np.float32)
    in_maps = []
    for core in range(NC_CORES):
        b, hg = core // 2, core % 2
        qs = slice(hg * 512, hg * 512 + 512)
        ks = slice(C + hg * 512, C + hg * 512 + 512)
        vs = slice(2 * C + hg * 512, 2 * C + hg * 512 + 512)
        w_c = np.concatenate(
            [attn_w[:, qs], attn_w[:, ks], attn_w[:, vs]], axis=1
        )
        in_maps.append(
            {
                "xT": np.ascontiguousarray(x[b].T),
                "w": np.ascontiguousarray(w_c),
                "wp": np.ascontiguousarray(
                    proj_w[hg * 512:hg * 512 + 512, :]
                ).astype(ml_dtypes.bfloat16),
                "bqk": np.ascontiguousarray(
                    np.concatenate([attn_b[qs], attn_b[ks]]).reshape(8, 128).T
                ),
                "bv": np.ascontiguousarray(
                    np.broadcast_to(attn_b[vs][None, :], (128, 512))
                ),
                "maskadd": maskadd,
                "ident": ident,
            }
        )

    res = run_bass_kernel_spmd(
        nc, in_maps, core_ids=list(range(NC_CORES)), trace=TRACE
    )
    LAST_RESULT = res

    out = np.empty((B, T, C), dtype=np.float32)
    for b in range(B):
        out[b] = (
            np.asarray(res.results[2 * b]["partial"], dtype=np.float32)
            + np.asarray(res.results[2 * b + 1]["partial"], dtype=np.float32)
            + proj_b[None, :]
        )
    return out


# revision 26
# speedup vs baseline: 1.0012x; 1.0012x over previous
"""Causal multi-head attention block (B=4, T=2048, C=1024, H=16) on 8 NeuronCores.

Sharding: core c = 2*b + hg handles batch b, head-group hg (8 heads).
Data parallel over B, tensor parallel over heads: qkv weights column-split,
proj weights row-split; each core emits a partial projection output which the
host sums per batch (plus proj bias).

Per-core pipeline (v2): single flat instruction stream that keeps the PE dense.
  - qkv projections produce qT/kT in bf16 [128, 2048] (head dims on
    partitions) and v token-major bf16 with a ones column.
  - attention runs per (head, 512-query quarter): scoresT tiles (bf16
    matmul, K padded 64->128 to keep the PE clock-gate warm), exp on ACT
    over cj-PAIRS packed side by side in one [128,1024] PSUM slab (halves
    the ACT instruction count), causal diag masked by 0/1 bf16 multiply,
    AV accumulates [65, 512] in PSUM where the ones column yields row sums.
  - the scores->exp->AV chain is software-pipelined one unit deep ACROSS
    quarter/head boundaries, and the NEXT pair's qkT projection matmuls are
    interleaved as fill work so the PE never idles on ACT latency.
  - normalization per quarter: reciprocal of row sums via DRAM bounce
    (fold [1,512] across partitions), broadcast, multiply -> yT bf16.
  - output projection in bf16 against row-slice of proj_w -> partial
    [2048, 1024] fp32, host sums pairs + proj bias.
"""
import numpy as np
import ml_dtypes

import concourse.bacc as bacc
import concourse.mybir as mybir
import concourse.tile as tile
from concourse.bass_utils import run_bass_kernel_spmd

B, T, C, H, D = 4, 2048, 1024, 16, 64
NC_CORES = 8
HPC = H // 2          # heads per core = 8
CW = 3 * C // 2       # packed local qkv width = 1536
F32 = mybir.dt.float32
F32R = mybir.dt.float32r
BF16 = mybir.dt.bfloat16

NT = T // 128         # 16 token tiles
NCC = C // 128        # 8 contraction chunks
NQ = 4                # query quarters per head
QW = 512              # quarter width

TRACE = False          # test.py sets True to profile
LAST_RESULT = None     # BassKernelResults of the last run (for test.py)

_DONE = object()       # generator-exhausted sentinel (yields return None)
_cached_nc = None


def _build():
    global _cached_nc
    if _cached_nc is not None:
        return _cached_nc

    nc = bacc.Bacc("TRN2", debug=False)

    xT_d = nc.dram_tensor("xT", [C, T], F32R, kind="ExternalInput")
    w_d = nc.dram_tensor("w", [C, CW], F32R, kind="ExternalInput")
    wp_d = nc.dram_tensor("wp", [C // 2, C], BF16, kind="ExternalInput")
    bqk_d = nc.dram_tensor("bqk", [128, 8], F32, kind="ExternalInput")
    bv_d = nc.dram_tensor("bv", [128, 512], F32, kind="ExternalInput")
    maskadd_d = nc.dram_tensor("maskadd", [128, 128], F32, kind="ExternalInput")
    ident_d = nc.dram_tensor("ident", [128, 128], F32, kind="ExternalInput")
    out_d = nc.dram_tensor("partial", [T, C], BF16, kind="ExternalOutput")

    with tile.TileContext(nc) as tc:
        with (
            tc.tile_pool(name="const", bufs=1) as const,
            tc.tile_pool(name="dramp", bufs=4, space="DRAM") as dramp,
            tc.tile_pool(name="ps", bufs=1, space="PSUM") as psp,
            tc.tile_pool(name="expp", bufs=2) as expp,
            tc.tile_pool(name="nrm", bufs=2) as nrm,
            tc.tile_pool(name="outp", bufs=4) as outp,
        ):
            # small consts first so the warmup matmuls can start immediately
            identf = const.tile([128, 128], F32)
            nc.sync.dma_start(identf[:], ident_d.ap())
            maskaddf = const.tile([128, 128], F32)
            nc.sync.dma_start(maskaddf[:], maskadd_d.ap())
            bqk = const.tile([128, 8], F32)
            nc.sync.dma_start(bqk[:], bqk_d.ap())
            bv = const.tile([128, 512], F32)
            nc.sync.dma_start(bv[:], bv_d.ap())

            # q/k activations, transposed, bf16: row = head dim (2 heads per
            # tile), col = token. jt 0-3 hold q, 4-7 hold k.
            qkT = [const.tile([128, T], BF16, name=f"qkT{j}") for j in range(8)]
            # zero-padded per-head kT staging: scores matmuls run with K=128
            # (half real head dims, half zeros) to keep the PE activity
            # monitor (clock gate) seeing full-K matmuls.
            kpad = [const.tile([128, T], BF16, name=f"kpad{s}") for s in range(2)]
            nc.vector.memset(kpad[0][:].bitcast(F32), 0.0)
            nc.vector.memset(kpad[1][:].bitcast(F32), 0.0)
            # v with ones column, token-major: v_aug[p, tt, h, d]
            v_aug = const.tile([128, NT, HPC, D + 1], BF16)
            nc.vector.memset(v_aug[:, :, :, D:D + 1], 1.0)
            # normalized attention output, bf16, row = head dim (2 heads)
            yT_sb = [const.tile([128, T], BF16, name=f"yT{k}") for k in range(4)]
            wp_sb = const.tile([128, 4, C], BF16)

            # qkv weight tiles: streamed through a 4-deep pool, prefetched
            # one attention pair ahead of their consuming fill matmuls
            wjt = {}

            # ---- mm tags: two [128,512] PSUM accumulators shared by the qkv
            # projections, v, warmup and P4.  scq: two [128,1024] score slabs.
            # yTq: two [65,512] AV accumulators.  2 + 4 + 2 = 8 banks.
            def mm_tile(i, name):
                return psp.tile([128, 512], F32, tag=f"mm{i}", bufs=1, name=name)

            with (
                tc.tile_pool(name="wjtp", bufs=4) as wjtp,
                tc.tile_pool(name="xp", bufs=1) as xp,
            ):
                def fetch_wjt(jt):
                    t = wjtp.tile(
                        [128, NCC, 128], F32R, tag="wjt", bufs=4, name=f"wjt{jt}"
                    )
                    fetch_wjt_split(jt, t)

                # first-needed weights and x, split fine and issued in consume
                # order so no single transfer serializes a queue for long.
                def fetch_wjt_split(jt, t):
                    for s in range(4):
                        nc.sync.dma_start(
                            t[:, 2 * s:2 * s + 2, :],
                            w_d.ap()[2 * s * 128:(2 * s + 2) * 128,
                                     jt * 128:(jt + 1) * 128].rearrange(
                                "(cc p) j -> p cc j", p=128
                            ),
                        )
                    wjt[jt] = t

                wjt0_t = wjtp.tile([128, NCC, 128], F32R, tag="wjt", bufs=4,
                                   name="wjt0")
                wjt4_t = wjtp.tile([128, NCC, 128], F32R, tag="wjt", bufs=4,
                                   name="wjt4")
                fetch_wjt_split(0, wjt0_t)
                xT = [xp.tile([128, T], F32R, name=f"xT{i}") for i in range(NCC)]

                def dma_x(cc, qtr):
                    nc.sync.dma_start(
                        xT[cc][:, qtr * 512:(qtr + 1) * 512],
                        xT_d.ap()[cc * 128:(cc + 1) * 128,
                                  qtr * 512:(qtr + 1) * 512],
                    )

                # cc-major for the first half: matches the qk cc-loop's
                # consume order so compute streams right behind the DMA
                for cc in range(NCC):
                    dma_x(cc, 0)
                    dma_x(cc, 1)
                fetch_wjt_split(4, wjt4_t)
                for cc in range(NCC):
                    dma_x(cc, 2)
                    dma_x(cc, 3)

                # warm the PE clock gate during the DMA wait: fp32 matmuls on
                # the identity tile (no DVE dependency, first DMA only)
                warm_ps = mm_tile(0, "warm")
                for wi in range(36):
                    nc.tensor.matmul(
                        warm_ps[:, 0:128],
                        lhsT=identf[:],
                        rhs=identf[:],
                        start=True,
                        stop=True,
                        skip_group_check=True,
                    )
                ident = const.tile([128, 128], BF16)
                nc.vector.tensor_copy(ident[:], identf[:])
                maskadd = const.tile([128, 128], BF16)
                nc.vector.tensor_copy(maskadd[:], maskaddf[:])

                def qk_tile_steps(jt):
                    """Generator: one PE matmul (or copy flush) per next()."""
                    for half in range(2):
                        pss = [mm_tile(i, f"qk{jt}_{half}_{i}") for i in range(2)]
                        for cc in range(NCC):
                            for i in range(2):
                                tck = half * 2 + i
                                nc.tensor.matmul(
                                    pss[i][:],
                                    lhsT=wjt[jt][:, cc, :],
                                    rhs=xT[cc][:, tck * 512:(tck + 1) * 512],
                                    start=(cc == 0),
                                    stop=(cc == NCC - 1),
                                )
                                yield
                        for i in range(2):
                            tck = half * 2 + i
                            nc.vector.tensor_scalar_add(
                                qkT[jt][:, tck * 512:(tck + 1) * 512],
                                pss[i][:],
                                bqk[:, jt:jt + 1],
                            )
                        yield

                def emit_all(gen):
                    for _ in gen:
                        pass

                # q,k for pair 0 (heads 0,1) directly
                emit_all(qk_tile_steps(0))
                emit_all(qk_tile_steps(4))

                with tc.tile_pool(name="wvp", bufs=1) as wvp:
                    wv = wvp.tile([128, NCC, 512], F32R)
                    for s in range(4):
                        nc.sync.dma_start(
                            wv[:, 2 * s:2 * s + 2, :],
                            w_d.ap()[2 * s * 128:(2 * s + 2) * 128,
                                     1024:1536].rearrange(
                                "(cc p) j -> p cc j", p=128
                            ),
                        )
                    # pair 0's fill weights stream in behind wv
                    fetch_wjt(1)
                    fetch_wjt(5)
                    # v: token-major, fused bias-add + bf16 pack on DVE
                    for tt in range(NT):
                        ps = mm_tile(tt % 2, f"v{tt}")
                        for cc in range(NCC):
                            nc.tensor.matmul(
                                ps[:],
                                lhsT=xT[cc][:, tt * 128:(tt + 1) * 128],
                                rhs=wv[:, cc, :],
                                start=(cc == 0),
                                stop=(cc == NCC - 1),
                            )
                        nc.vector.tensor_add(
                            v_aug[:, tt, :, 0:D],
                            ps[:].rearrange("p (h d) -> p h d", h=HPC),
                            bv[:].rearrange("p (h d) -> p h d", h=HPC),
                        )

                # proj weights arrive during attention
                nc.sync.dma_start(
                    wp_sb[:], wp_d.ap().rearrange("(kc p) n -> p kc n", p=128)
                )

                # ---------------- attention: flat pipelined stream ----------
                pend = [None]  # deferred (av_closures, fin_closure)

                def flush_pend():
                    if pend[0] is not None:
                        avs, fin = pend[0]
                        for av in avs:
                            av()
                        if fin is not None:
                            fin()
                        pend[0] = None

                def make_fin(yTq_t, hh, off, qlo):
                    def fin():
                        sums = nrm.tile([1, QW], F32, tag="sums", bufs=2)
                        nc.vector.tensor_copy(sums[:], yTq_t[D:D + 1, :])
                        ynum = nrm.tile([64, QW], F32, tag="ynum", bufs=3)
                        nc.vector.tensor_copy(ynum[:], yTq_t[0:D, :])
                        s_dram = dramp.tile([1, QW], F32, tag="sd")
                        nc.sync.dma_start(s_dram[:], sums[:])
                        srb = nrm.tile([128, QW // 128], F32, tag="srb", bufs=2)
                        nc.sync.dma_start(
                            srb[:],
                            s_dram[:].rearrange("o (p f) -> (o p) f", p=128),
                        )
                        rcp = nrm.tile([128, QW // 128], F32, tag="rcp", bufs=2)
                        nc.vector.reciprocal(rcp[:], srb[:])
                        r_dram = dramp.tile([1, QW], F32, tag="rd")
                        nc.sync.dma_start(
                            r_dram[:].rearrange("o (p f) -> (o p) f", p=128),
                            rcp[:],
                        )
                        rb = nrm.tile([64, QW], F32, tag="rb", bufs=2)
                        nc.sync.dma_start(
                            rb[:], r_dram[:].to_broadcast((64, QW))
                        )
                        nc.vector.tensor_mul(
                            yT_sb[hh][off:off + 64, qlo:qlo + QW],
                            ynum[:],
                            rb[:],
                        )
                    return fin

                def attention_pair(p, fills):
                    for h in (2 * p, 2 * p + 1):
                        off = 64 * (h % 2)
                        jq = h // 2
                        kp = kpad[h % 2]
                        nc.vector.tensor_copy(
                            kp[off:off + 64, :], qkT[4 + h // 2][off:off + 64, :]
                        )
                        for q in range(NQ):
                            qlo = q * QW
                            ncj = 4 * q + 4
                            yTq_t = psp.tile(
                                [D + 1, QW], F32, tag="yTq", bufs=2,
                                name=f"yTq{h}_{q}",
                            )
                            fin = make_fin(yTq_t, h // 2, off, qlo)
                            for cjA in range(0, ncj, 2):
                                # pack the cj pair side by side in one slab:
                                # each matmul output stays inside one PSUM bank
                                blocks = []
                                pack = 0
                                for cj in (cjA, cjA + 1):
                                    i0 = cj * 128
                                    s0 = max(i0, qlo)
                                    wdt = qlo + QW - s0
                                    if cj == cjA + 1 and pack == 512 and wdt == 512:
                                        pk = 512
                                    else:
                                        pk = pack if pack % 512 == 0 or pack + wdt <= 512 else 512
                                    blocks.append((cj, i0, s0, wdt, pk))
                                    pack = pk + wdt
                                scq = psp.tile(
                                    [128, 1024], F32, tag="scq", bufs=2,
                                    name=f"scq{h}_{q}_{cjA}",
                                )
                                expT = expp.tile(
                                    [128, 1024], BF16, tag="expT", bufs=2,
                                    name=f"expT{h}_{q}_{cjA}",
                                )
                                for cj, i0, s0, wdt, pk in blocks:
                                    diag = i0 >= qlo
                                    nc.tensor.matmul(
                                        scq[:, pk:pk + wdt],
                                        lhsT=kp[:, i0:i0 + 128],
                                        rhs=qkT[jq][:, s0:s0 + wdt],
                                        start=True,
                                        stop=not diag,
                                        skip_group_check=True,
                                    )
                                    if diag:
                                        # causal mask folded into the PSUM
                                        # group: += maskadd^T (-1e30 above
                                        # diag), exp then yields exact zeros
                                        nc.tensor.matmul(
                                            scq[:, pk:pk + 128],
                                            lhsT=maskadd[:],
                                            rhs=ident[:],
                                            start=False,
                                            stop=True,
                                            skip_group_check=True,
                                        )
                                # fill: next pair's qkv projection matmuls
                                for _ in range(2):
                                    if fills is not None:
                                        if next(fills, _DONE) is _DONE:
                                            fills = None
                                ew = blocks[-1][4] + blocks[-1][3]
                                nc.scalar.activation(
                                    expT[:, 0:ew],
                                    scq[:, 0:ew],
                                    mybir.ActivationFunctionType.Exp,
                                )
                                flush_pend()
                                avs = []
                                for cj, i0, s0, wdt, pk in blocks:
                                    def av(cj=cj, s0=s0, wdt=wdt, pk=pk,
                                           expT=expT, yTq_t=yTq_t, h=h,
                                           qlo=qlo, ncj=ncj):
                                        nc.tensor.matmul(
                                            yTq_t[:, s0 - qlo:s0 - qlo + wdt],
                                            lhsT=v_aug[:, cj, h, :],
                                            rhs=expT[:, pk:pk + wdt],
                                            start=(cj == 0),
                                            stop=(cj == ncj - 1),
                                            skip_group_check=True,
                                        )
                                    avs.append(av)
                                pend[0] = (
                                    avs,
                                    fin if cjA == ncj - 2 else None,
                                )
                    return fills

                def qk_pair_fills(p):
                    def chain():
                        yield from qk_tile_steps(p + 1)
                        yield from qk_tile_steps(p + 5)
                    return chain()

                for p in range(3):
                    if p + 2 <= 3:
                        fetch_wjt(p + 2)
                        fetch_wjt(p + 6)
                    leftover = attention_pair(p, qk_pair_fills(p))
                    if leftover is not None:
                        emit_all(leftover)

            # xT freed; pair 3's fill work: P4 partial sums over kc 0-2 for
            # the first 16 output chunks, staged to SBUF (kc3 needs pair 3)
            NPRE = 16
            psb = {}
            with tc.tile_pool(name="psbp", bufs=1) as psbp:
                def p4_prefill_steps():
                    for n in range(NPRE):
                        ci, nck = n // 2, n % 2
                        po = mm_tile(n % 2, f"pop{n}")
                        for ki in range(3):
                            nc.tensor.matmul(
                                po[:],
                                lhsT=yT_sb[ki][:, ci * 128:(ci + 1) * 128],
                                rhs=wp_sb[:, ki, nck * 512:(nck + 1) * 512],
                                start=(ki == 0),
                                stop=(ki == 2),
                            )
                            yield
                        t = psbp.tile([128, 512], F32, name=f"psb{n}")
                        nc.vector.tensor_copy(t[:], po[:])
                        psb[n] = t
                        yield

                leftover = attention_pair(3, p4_prefill_steps())
                if leftover is not None:
                    emit_all(leftover)
                flush_pend()

                # ------------- P4: output projection (partial) -------------
                # prefilled chunks: one kc3 matmul + DVE add of the staged
                # partials; DMA flows immediately
                for n in range(NPRE):
                    ci, nck = n // 2, n % 2
                    po = mm_tile(n % 2, f"pof{n}")
                    nc.tensor.matmul(
                        po[:],
                        lhsT=yT_sb[3][:, ci * 128:(ci + 1) * 128],
                        rhs=wp_sb[:, 3, nck * 512:(nck + 1) * 512],
                        start=True,
                        stop=True,
                    )
                    osb = outp.tile([128, 512], BF16, tag="osb", bufs=4)
                    nc.vector.tensor_add(osb[:], po[:], psb[n][:])
                    nc.sync.dma_start(
                        out_d.ap()[ci * 128:(ci + 1) * 128,
                                   nck * 512:(nck + 1) * 512],
                        osb[:],
                    )
                # remaining chunks: full 4-matmul accumulation, copies on ACT
                # (free after exp) so DVE keeps doing the adds above
                for n in range(NPRE, 2 * NT):
                    ci, nck = n // 2, n % 2
                    po = mm_tile(n % 2, f"po{n}")
                    for ki in range(4):
                        nc.tensor.matmul(
                            po[:],
                            lhsT=yT_sb[ki][:, ci * 128:(ci + 1) * 128],
                            rhs=wp_sb[:, ki, nck * 512:(nck + 1) * 512],
                            start=(ki == 0),
                            stop=(ki == 3),
                        )
                    osb = outp.tile([128, 512], BF16, tag="osb", bufs=4)
                    nc.scalar.copy(osb[:], po[:])
                    nc.sync.dma_start(
                        out_d.ap()[ci * 128:(ci + 1) * 128,
                                   nck * 512:(nck + 1) * 512],
                        osb[:],
                    )

    nc.compile()
    _cached_nc = nc
    return nc


def kernel(x, attn_w, attn_b, proj_w, proj_b):
    global LAST_RESULT
    x = np.asarray(x, dtype=np.float32)
    attn_w = np.asarray(attn_w, dtype=np.float32)
    attn_b = np.asarray(attn_b, dtype=np.float32)
    proj_w = np.asarray(proj_w, dtype=np.float32)
    proj_b = np.asarray(proj_b, dtype=np.float32)

    nc = _build()

    # additive causal mask, pre-transposed for the matmul trick:
    # maskadd[c, p] lands as -1e30 on scoresT[key p, query c] for c < p
    maskadd = np.triu(np.full((128, 128), -1e30, dtype=np.float32), 1)
    ident = np.eye(128, dtype=np.float32)
    in_maps = []
    for core in range(NC_CORES):
        b, hg = core // 2, core % 2
        qs = slice(hg * 512, hg * 512 + 512)
        ks = slice(C + hg * 512, C + hg * 512 + 512)
        vs = slice(2 * C + hg * 512, 2 * C + hg * 512 + 512)
        w_c = np.concatenate(
            [attn_w[:, qs], attn_w[:, ks], attn_w[:, vs]], axis=1
        )
        in_maps.append(
            {
                "xT": np.ascontiguousarray(x[b].T),
                "w": np.ascontiguousarray(w_c),
                "wp": np.ascontiguousarray(
                    proj_w[hg * 512:hg * 512 + 512, :]
                ).astype(ml_dtypes.bfloat16),
                "bqk": np.ascontiguousarray(
                    np.concatenate([attn_b[qs], attn_b[ks]]).reshape(8, 128).T
                ),
                "bv": np.ascontiguousarray(
                    np.broadcast_to(attn_b[vs][None, :], (128, 512))
                ),
                "maskadd": maskadd,
                "ident": ident,
            }
        )

    res = run_bass_kernel_spmd(
        nc, in_maps, core_ids=list(range(NC_CORES)), trace=TRACE
    )
    LAST_RESULT = res

    out = np.empty((B, T, C), dtype=np.float32)
    for b in range(B):
        out[b] = (
            np.asarray(res.results[2 * b]["partial"], dtype=np.float32)
            + np.asarray(res.results[2 * b + 1]["partial"], dtype=np.float32)
            + proj_b[None, :]
        )
    return out
